# revision 4
# baseline (speedup 1.0000x reference)
"""Gated multi-head attention on 8 NeuronCores.

Sharding (hardcoded): core c -> (batch b = c // 4, head-group g = c % 4).
Data-parallel over B=2, tensor-parallel over the 16 heads in groups of 4.
Each core computes its 4 heads' attention plus the corresponding slice of
the output projection; the host sums the 4 head-group partials per batch
and adds the output bias.

Per-core kernel (all matmuls fp32r unless noted):
  qT[256,2048] = (Wq_slice.T).T @ x_q.T   (gate sigmoid/sqrt(D) + bias folded
                                           into the PSUM->SBUF eviction)
  kT[256,2048], v[2048,256] likewise (v in natural [pos, dim] layout, bf16,
                                      with a ones-column appended per head)
  per head h, per 512-wide query block:
    S^T[k,q] = kT_h.T-chunks @ qT_h      (K=64 contraction)
    P^T = exp(S^T)                       (no max-subtraction: logits ~ +-4)
    acc[q,65] = P^T-chunks.T @ [V_h | 1] (bf16; col 64 = softmax denominator)
    A[q, h*64:...] = acc[:, :64] * recip(acc[:, 64])
  y[q,1024] = A @ Wo_slice.T  via PE-transpose of A then fp32r matmul
"""

import math
from contextlib import ExitStack

import numpy as np

import concourse.bass as bass
import concourse.tile as tile
from concourse import mybir
from concourse.bass_utils import run_bass_kernel_spmd
from concourse.masks import make_identity

B = 2
N = 2048
E = 1024
H = 16
D = 64
NCORES = 8
GROUPS = NCORES // B      # head-groups per batch
HG = H // GROUPS          # heads per core
DH = HG * D               # 256 head-dims per core
P = 128

F32 = mybir.dt.float32
F32R = mybir.dt.float32r
BF16 = mybir.dt.bfloat16
AF = mybir.ActivationFunctionType

TRACE = False
LAST_RESULTS = None


def _split_drain_waits(nc):
    """The installed walrus build accepts only ONE sync-wait per instruction
    (one NEURON_ISA_TPB_EVENTS slot), but Tile emits several on drains,
    matmuls, etc.  Hoist all but the last wait onto dedicated single-wait
    NOPs ahead of the instruction on the same engine (the lowering newer
    walrus performs itself)."""
    n = 0
    for fn in nc.m.functions:
        for bb in fn.blocks:
            insts = bb.instructions
            idx = 0
            while idx < len(insts):
                inst = insts[idx]
                si = inst.sync_info
                if si is not None and len(si.on_wait) > 1:
                    waits = list(si.on_wait)
                    nops = []
                    for w in waits[:-1]:
                        n += 1
                        nop = mybir.InstNoOp(
                            name=f"waitsplit-{n}",
                            engine=inst.engine,
                            sync_info=mybir.SyncInfo(on_wait=[w], on_update=[]),
                            bass_nofuse=True,
                        )
                        nc.register_instruction(nop)
                        nops.append(nop)
                    inst.sync_info = mybir.SyncInfo(
                        on_wait=[waits[-1]], on_update=list(si.on_update))
                    insts[idx:idx] = nops
                    idx += len(nops)
                idx += 1
    return n


def _r(ap):
    return ap.bitcast(F32R)


def _build():
    nc = bass.Bass()
    xqT = nc.dram_tensor("xqT", [E, N], F32R, kind="ExternalInput")
    xkT = nc.dram_tensor("xkT", [E, N], F32R, kind="ExternalInput")
    xvT = nc.dram_tensor("xvT", [E, N], F32R, kind="ExternalInput")
    wqT = nc.dram_tensor("wqT", [E, DH], F32R, kind="ExternalInput")
    wkT = nc.dram_tensor("wkT", [E, DH], F32R, kind="ExternalInput")
    wvT = nc.dram_tensor("wvT", [E, DH], F32R, kind="ExternalInput")
    woB = nc.dram_tensor("woB", [DH, E], F32R, kind="ExternalInput")
    qscale = nc.dram_tensor("qscale", [DH], F32, kind="ExternalInput")
    qbias = nc.dram_tensor("qbias", [DH], F32, kind="ExternalInput")
    kbias = nc.dram_tensor("kbias", [DH], F32, kind="ExternalInput")
    vbias = nc.dram_tensor("vbias", [DH], F32, kind="ExternalInput")
    y = nc.dram_tensor("y", [N, E], F32, kind="ExternalOutput")

    KC = E // P            # 8 contraction chunks over the embed dim
    MC = DH // P           # 2 partition chunks over this core's head dims
    NB = N // 512          # 4 query blocks
    KB = N // P            # 16 key-position chunks

    with ExitStack() as ctx:
        tc = ctx.enter_context(tile.TileContext(nc))
        const = ctx.enter_context(tc.tile_pool(name="const", bufs=1))
        xpool = ctx.enter_context(tc.tile_pool(name="xpool", bufs=KC))
        wpool = ctx.enter_context(tc.tile_pool(name="wpool", bufs=KC))
        wopool = ctx.enter_context(tc.tile_pool(name="wopool", bufs=MC))
        qkpool = ctx.enter_context(tc.tile_pool(name="qkpool", bufs=MC))
        vpool = ctx.enter_context(tc.tile_pool(name="vpool", bufs=KB))
        ptpool = ctx.enter_context(tc.tile_pool(name="ptpool", bufs=20))
        apool = ctx.enter_context(tc.tile_pool(name="apool", bufs=6))
        atpool = ctx.enter_context(tc.tile_pool(name="atpool", bufs=4))
        ypool = ctx.enter_context(tc.tile_pool(name="ypool", bufs=3))
        spool = ctx.enter_context(tc.tile_pool(name="spool", bufs=8))
        ps = ctx.enter_context(tc.tile_pool(name="ps", bufs=5, space="PSUM"))
        pss = ctx.enter_context(tc.tile_pool(name="pss", bufs=3, space="PSUM"))

        ident = const.tile([P, P], F32)
        make_identity(nc, ident)

        qs_sb = const.tile([P, MC], F32, name="qs")
        nc.sync.dma_start(out=qs_sb, in_=qscale[:].rearrange("(c p) -> p c", p=P))
        qb_sb = const.tile([P, MC], F32, name="qb")
        nc.sync.dma_start(out=qb_sb, in_=qbias[:].rearrange("(c p) -> p c", p=P))
        kb_sb = const.tile([P, MC], F32, name="kb")
        nc.sync.dma_start(out=kb_sb, in_=kbias[:].rearrange("(c p) -> p c", p=P))
        vb_ap = vbias[:]
        vb_bc = const.tile([P, DH], F32, name="vb")
        nc.gpsimd.dma_start(out=vb_bc, in_=bass.AP(
            tensor=vb_ap.tensor, offset=vb_ap.offset, ap=[[0, P]] + vb_ap.ap))

        def load_w(w_dram, tag):
            chunks = []
            for kc in range(KC):
                t = wpool.tile([P, DH], F32R, name=tag)
                nc.sync.dma_start(out=t, in_=w_dram[kc * P:(kc + 1) * P, :])
                chunks.append(t)
            return chunks

        wq_c = load_w(wqT, "wq")
        wk_c = load_w(wkT, "wk")
        wv_c = load_w(wvT, "wv")
        wo_sb = []
        for c in range(MC):
            t = wopool.tile([P, E], F32R, name="wo")
            nc.sync.dma_start(out=t, in_=woB[c * P:(c + 1) * P, :])
            wo_sb.append(t)

        def load_x(x_dram):
            xs = []
            for kc in range(KC):
                t = xpool.tile([P, N], F32R, name="xs")
                nc.sync.dma_start(out=t, in_=x_dram[kc * P:(kc + 1) * P, :])
                xs.append(t)
            return xs

        # --- transposed projections: out[c][dd, n] ---
        def proj_T(xs, w_c, tag, scale_sb, bias_sb):
            outs = []
            for c in range(MC):
                o = qkpool.tile([P, N], F32R, name=tag)
                outs.append(o)
                for nb in range(NB):
                    pt = ps.tile([P, 512], F32, name="ps")
                    for kc in range(KC):
                        nc.tensor.matmul(
                            pt,
                            lhsT=w_c[kc][:, c * P:(c + 1) * P],
                            rhs=xs[kc][:, nb * 512:(nb + 1) * 512],
                            start=(kc == 0), stop=(kc == KC - 1))
                    nc.scalar.activation(
                        out=o[:, nb * 512:(nb + 1) * 512], in_=pt,
                        func=AF.Identity,
                        bias=bias_sb[:, c:c + 1],
                        scale=scale_sb[:, c:c + 1] if scale_sb is not None else 1.0)
            return outs

        xq = load_x(xqT)
        qT = proj_T(xq, wq_c, "qt", qs_sb, qb_sb)
        xk = load_x(xkT)
        kT = proj_T(xk, wk_c, "kt", None, kb_sb)

        # --- v in natural [pos, dim] layout, bf16, ones column at dim 64 ---
        xv = load_x(xvT)
        v_sb = []
        for m in range(KB):
            vt = vpool.tile([P, HG, D + 1], BF16, name="vt")
            nc.gpsimd.memset(vt[:, :, D:D + 1], 1.0)
            pv = ps.tile([P, DH], F32, name="ps")
            for kc in range(KC):
                nc.tensor.matmul(
                    pv,
                    lhsT=xv[kc][:, m * P:(m + 1) * P],
                    rhs=wv_c[kc],
                    start=(kc == 0), stop=(kc == KC - 1))
            nc.vector.tensor_add(
                out=vt[:, :, 0:D],
                in0=pv.rearrange("p (h d) -> p h d", h=HG),
                in1=vb_bc.rearrange("p (h d) -> p h d", h=HG))
            v_sb.append(vt)

        # --- attention + output projection, one 512-wide query block at a time
        for qb in range(NB):
            a_tiles = [apool.tile([P, DH], F32, name="acc") for _ in range(4)]
            for h in range(HG):
                c, off = divmod(h * D, P)
                pts = []
                for kc in range(KB):
                    stp = ps.tile([P, 512], F32, name="ps")
                    nc.tensor.matmul(
                        stp,
                        lhsT=kT[c][off:off + D, kc * P:(kc + 1) * P],
                        rhs=qT[c][off:off + D, qb * 512:(qb + 1) * 512],
                        start=True, stop=True)
                    ptile = ptpool.tile([P, 512], BF16, name="pt")
                    nc.scalar.activation(out=ptile, in_=stp, func=AF.Exp)
                    pts.append(ptile)
                for ql in range(4):
                    av = pss.tile([P, D + 1], F32, name="pss")
                    for kc in range(KB):
                        nc.tensor.matmul(
                            av,
                            lhsT=pts[kc][:, ql * P:(ql + 1) * P],
                            rhs=v_sb[kc][:, h, :],
                            start=(kc == 0), stop=(kc == KB - 1))
                    rt = spool.tile([P, 1], F32, name="rt")
                    nc.vector.reciprocal(out=rt, in_=av[:, D:D + 1])
                    nc.vector.tensor_scalar_mul(
                        out=a_tiles[ql][:, h * D:(h + 1) * D],
                        in0=av[:, 0:D], scalar1=rt)
            for ql in range(4):
                at = []
                for c2 in range(MC):
                    tp = pss.tile([P, P], F32, name="pss")
                    nc.tensor.transpose(
                        tp, a_tiles[ql][:, c2 * P:(c2 + 1) * P], ident)
                    att = atpool.tile([P, P], F32R, name="att")
                    nc.vector.tensor_copy(out=att, in_=tp)
                    at.append(att)
                yt = ypool.tile([P, E], F32, name="yt")
                for nn in range(2):
                    py = ps.tile([P, 512], F32, name="ps")
                    for c2 in range(MC):
                        nc.tensor.matmul(
                            py, lhsT=at[c2],
                            rhs=wo_sb[c2][:, nn * 512:(nn + 1) * 512],
                            start=(c2 == 0), stop=(c2 == MC - 1))
                    nc.vector.tensor_copy(
                        out=yt[:, nn * 512:(nn + 1) * 512], in_=py)
                q0 = qb * 512 + ql * P
                nc.sync.dma_start(out=y[q0:q0 + P, :], in_=yt)

    _split_drain_waits(nc)
    return nc


_CACHE = {}


def _get_nc():
    if "nc" not in _CACHE:
        _CACHE["nc"] = _build()
    return _CACHE["nc"]


def kernel(query, key, value, Wq, bq, Wk, bk, Wv, bv, Wo, bo, gate):
    global LAST_RESULTS
    query = np.asarray(query, np.float32)
    key = np.asarray(key, np.float32)
    value = np.asarray(value, np.float32)
    Wq = np.asarray(Wq, np.float32)
    Wk = np.asarray(Wk, np.float32)
    Wv = np.asarray(Wv, np.float32)
    Wo = np.asarray(Wo, np.float32)
    bq = np.asarray(bq, np.float32)
    bk = np.asarray(bk, np.float32)
    bv = np.asarray(bv, np.float32)
    bo = np.asarray(bo, np.float32)
    gate = np.asarray(gate, np.float32)

    scale_h = (1.0 / (1.0 + np.exp(-gate.astype(np.float64)))
               / math.sqrt(D)).astype(np.float32)

    xq_b = [np.ascontiguousarray(query[b].T) for b in range(B)]
    xk_b = [np.ascontiguousarray(key[b].T) for b in range(B)]
    xv_b = [np.ascontiguousarray(value[b].T) for b in range(B)]

    in_maps = []
    for core in range(NCORES):
        b, g = divmod(core, GROUPS)
        rows = slice(g * DH, (g + 1) * DH)
        qs = np.repeat(scale_h[g * HG:(g + 1) * HG], D)
        in_maps.append({
            "xqT": xq_b[b], "xkT": xk_b[b], "xvT": xv_b[b],
            "wqT": np.ascontiguousarray(Wq[rows].T),
            "wkT": np.ascontiguousarray(Wk[rows].T),
            "wvT": np.ascontiguousarray(Wv[rows].T),
            "woB": np.ascontiguousarray(Wo[:, rows].T),
            "qscale": np.ascontiguousarray(qs),
            "qbias": np.ascontiguousarray(bq[rows] * qs),
            "kbias": np.ascontiguousarray(bk[rows]),
            "vbias": np.ascontiguousarray(bv[rows]),
        })

    res = run_bass_kernel_spmd(_get_nc(), in_maps, list(range(NCORES)),
                               trace=TRACE)
    LAST_RESULTS = res
    out = np.empty((B, N, E), np.float32)
    for b in range(B):
        acc = res.results[b * GROUPS]["y"].astype(np.float32).copy()
        for g in range(1, GROUPS):
            acc += res.results[b * GROUPS + g]["y"]
        out[b] = acc + bo
    return out


# revision 5
# speedup vs baseline: 1.1496x; 1.1496x over previous
"""Gated multi-head attention on 8 NeuronCores.

Sharding (hardcoded): core c -> (batch b = c // 4, head-group g = c % 4).
Data-parallel over B=2, tensor-parallel over the 16 heads in groups of 4.
Each core computes its 4 heads' attention plus the corresponding slice of
the output projection; the host sums the 4 head-group partials per batch
and adds the output bias.

Per-core kernel (all matmuls fp32r unless noted):
  qT[256,2048] = (Wq_slice.T).T @ x_q.T   (gate sigmoid/sqrt(D) + bias folded
                                           into the PSUM->SBUF eviction)
  kT[256,2048], v[2048,256] likewise (v in natural [pos, dim] layout, bf16,
                                      with a ones-column appended per head)
  per head h, per 512-wide query block:
    S^T[k,q] = kT_h.T-chunks @ qT_h      (K=64 contraction)
    P^T = exp(S^T)                       (no max-subtraction: logits ~ +-4)
    acc[q,65] = P^T-chunks.T @ [V_h | 1] (bf16; col 64 = softmax denominator)
    A[q, h*64:...] = acc[:, :64] * recip(acc[:, 64])
  y[q,1024] = A @ Wo_slice.T  via PE-transpose of A then fp32r matmul
"""

import math
from contextlib import ExitStack

import numpy as np

import concourse.bass as bass
import concourse.tile as tile
from concourse import mybir
from concourse.bass_utils import run_bass_kernel_spmd
from concourse.masks import make_identity

B = 2
N = 2048
E = 1024
H = 16
D = 64
NCORES = 8
GROUPS = NCORES // B      # head-groups per batch
HG = H // GROUPS          # heads per core
DH = HG * D               # 256 head-dims per core
P = 128

F32 = mybir.dt.float32
F32R = mybir.dt.float32r
BF16 = mybir.dt.bfloat16
AF = mybir.ActivationFunctionType

TRACE = False
LAST_RESULTS = None


def _split_drain_waits(nc):
    """The installed walrus build accepts only ONE sync-wait per instruction
    (one NEURON_ISA_TPB_EVENTS slot), but Tile emits several on drains,
    matmuls, etc.  Hoist all but the last wait onto dedicated single-wait
    NOPs ahead of the instruction on the same engine (the lowering newer
    walrus performs itself)."""
    n = 0
    for fn in nc.m.functions:
        for bb in fn.blocks:
            insts = bb.instructions
            idx = 0
            while idx < len(insts):
                inst = insts[idx]
                si = inst.sync_info
                if si is not None and len(si.on_wait) > 1:
                    waits = list(si.on_wait)
                    nops = []
                    for w in waits[:-1]:
                        n += 1
                        nop = mybir.InstNoOp(
                            name=f"waitsplit-{n}",
                            engine=inst.engine,
                            sync_info=mybir.SyncInfo(on_wait=[w], on_update=[]),
                            bass_nofuse=True,
                        )
                        nc.register_instruction(nop)
                        nops.append(nop)
                    inst.sync_info = mybir.SyncInfo(
                        on_wait=[waits[-1]], on_update=list(si.on_update))
                    insts[idx:idx] = nops
                    idx += len(nops)
                idx += 1
    return n


def _r(ap):
    return ap.bitcast(F32R)


def _build():
    nc = bass.Bass()
    xqT = nc.dram_tensor("xqT", [E, N], F32R, kind="ExternalInput")
    xkT = nc.dram_tensor("xkT", [E, N], F32R, kind="ExternalInput")
    xvT = nc.dram_tensor("xvT", [E, N], F32R, kind="ExternalInput")
    wqT = nc.dram_tensor("wqT", [E, DH], F32R, kind="ExternalInput")
    wkT = nc.dram_tensor("wkT", [E, DH], F32R, kind="ExternalInput")
    wvT = nc.dram_tensor("wvT", [E, DH], F32R, kind="ExternalInput")
    woB = nc.dram_tensor("woB", [DH, E], F32R, kind="ExternalInput")
    qscale = nc.dram_tensor("qscale", [DH], F32, kind="ExternalInput")
    qbias = nc.dram_tensor("qbias", [DH], F32, kind="ExternalInput")
    kbias = nc.dram_tensor("kbias", [DH], F32, kind="ExternalInput")
    vbias = nc.dram_tensor("vbias", [DH], F32, kind="ExternalInput")
    y = nc.dram_tensor("y", [N, E], F32, kind="ExternalOutput")

    KC = E // P            # 8 contraction chunks over the embed dim
    MC = DH // P           # 2 partition chunks over this core's head dims
    NB = N // 512          # 4 query blocks
    KB = N // P            # 16 key-position chunks

    with ExitStack() as ctx:
        tc = ctx.enter_context(tile.TileContext(nc))
        const = ctx.enter_context(tc.tile_pool(name="const", bufs=1))
        xpool = ctx.enter_context(tc.tile_pool(name="xpool", bufs=KC))
        wpool = ctx.enter_context(tc.tile_pool(name="wpool", bufs=KC))
        wopool = ctx.enter_context(tc.tile_pool(name="wopool", bufs=MC))
        qkpool = ctx.enter_context(tc.tile_pool(name="qkpool", bufs=MC))
        vpool = ctx.enter_context(tc.tile_pool(name="vpool", bufs=KB))
        ptpool = ctx.enter_context(tc.tile_pool(name="ptpool", bufs=18))
        apool = ctx.enter_context(tc.tile_pool(name="apool", bufs=10))
        atpool = ctx.enter_context(tc.tile_pool(name="atpool", bufs=4))
        ypool = ctx.enter_context(tc.tile_pool(name="ypool", bufs=2))
        spool = ctx.enter_context(tc.tile_pool(name="spool", bufs=8))
        ps = ctx.enter_context(tc.tile_pool(name="ps", bufs=3, space="PSUM"))
        pss = ctx.enter_context(tc.tile_pool(name="pss", bufs=2, space="PSUM"))

        ident = const.tile([P, P], F32)
        make_identity(nc, ident)

        qs_sb = const.tile([P, MC], F32, name="qs")
        nc.sync.dma_start(out=qs_sb, in_=qscale[:].rearrange("(c p) -> p c", p=P))
        qb_sb = const.tile([P, MC], F32, name="qb")
        nc.sync.dma_start(out=qb_sb, in_=qbias[:].rearrange("(c p) -> p c", p=P))
        kb_sb = const.tile([P, MC], F32, name="kb")
        nc.sync.dma_start(out=kb_sb, in_=kbias[:].rearrange("(c p) -> p c", p=P))
        vb_ap = vbias[:]
        vb_bc = const.tile([P, DH], F32, name="vb")
        nc.gpsimd.dma_start(out=vb_bc, in_=bass.AP(
            tensor=vb_ap.tensor, offset=vb_ap.offset, ap=[[0, P]] + vb_ap.ap))

        def load_w(w_dram, tag):
            chunks = []
            for kc in range(KC):
                t = wpool.tile([P, DH], F32R, name=tag)
                nc.sync.dma_start(out=t, in_=w_dram[kc * P:(kc + 1) * P, :])
                chunks.append(t)
            return chunks

        wq_c = load_w(wqT, "wq")
        wk_c = load_w(wkT, "wk")
        wv_c = load_w(wvT, "wv")
        wo_sb = []
        for c in range(MC):
            t = wopool.tile([P, E], F32R, name="wo")
            nc.sync.dma_start(out=t, in_=woB[c * P:(c + 1) * P, :])
            wo_sb.append(t)

        def load_x(x_dram):
            xs = []
            for kc in range(KC):
                t = xpool.tile([P, N], F32R, name="xs")
                nc.sync.dma_start(out=t, in_=x_dram[kc * P:(kc + 1) * P, :])
                xs.append(t)
            return xs

        # --- transposed projections: out[c][dd, n] ---
        def proj_T(xs, w_c, tag, scale_sb, bias_sb):
            outs = []
            for c in range(MC):
                o = qkpool.tile([P, N], BF16, name=tag)
                outs.append(o)
                for nb in range(NB):
                    pt = ps.tile([P, 1024], F32, name="ps")[:, :512]
                    for kc in range(KC):
                        nc.tensor.matmul(
                            pt,
                            lhsT=w_c[kc][:, c * P:(c + 1) * P],
                            rhs=xs[kc][:, nb * 512:(nb + 1) * 512],
                            start=(kc == 0), stop=(kc == KC - 1))
                    nc.scalar.activation(
                        out=o[:, nb * 512:(nb + 1) * 512], in_=pt,
                        func=AF.Identity,
                        bias=bias_sb[:, c:c + 1],
                        scale=scale_sb[:, c:c + 1] if scale_sb is not None else 1.0)
            return outs

        xq = load_x(xqT)
        qT = proj_T(xq, wq_c, "qt", qs_sb, qb_sb)
        xk = load_x(xkT)
        kT = proj_T(xk, wk_c, "kt", None, kb_sb)

        # --- v in natural [pos, dim] layout, bf16, ones column at dim 64 ---
        xv = load_x(xvT)
        v_sb = []
        for m in range(KB):
            vt = vpool.tile([P, HG, D + 1], BF16, name="vt")
            nc.gpsimd.memset(vt[:, :, D:D + 1], 1.0)
            pv = ps.tile([P, 1024], F32, name="ps")[:, :DH]
            for kc in range(KC):
                nc.tensor.matmul(
                    pv,
                    lhsT=xv[kc][:, m * P:(m + 1) * P],
                    rhs=wv_c[kc],
                    start=(kc == 0), stop=(kc == KC - 1))
            nc.vector.tensor_add(
                out=vt[:, :, 0:D],
                in0=pv.rearrange("p (h d) -> p h d", h=HG),
                in1=vb_bc.rearrange("p (h d) -> p h d", h=HG))
            v_sb.append(vt)

        # --- attention + output projection, one 1024-wide query block at a time
        for qb in range(N // 1024):
            a_tiles = [apool.tile([P, DH], F32, name="acc") for _ in range(8)]
            for h in range(HG):
                c, off = divmod(h * D, P)
                pts = []
                for kc in range(KB):
                    stp = ps.tile([P, 1024], F32, name="ps")
                    for qh in range(2):
                        nc.tensor.matmul(
                            stp[:, qh * 512:(qh + 1) * 512],
                            lhsT=kT[c][off:off + D, kc * P:(kc + 1) * P],
                            rhs=qT[c][off:off + D,
                                      qb * 1024 + qh * 512:qb * 1024 + (qh + 1) * 512],
                            start=True, stop=True)
                    ptile = ptpool.tile([P, 1024], BF16, name="pt")
                    nc.scalar.activation(out=ptile, in_=stp, func=AF.Exp)
                    pts.append(ptile)
                for ql in range(8):
                    av = pss.tile([P, D + 1], F32, name="pss")
                    for kc in range(KB):
                        nc.tensor.matmul(
                            av,
                            lhsT=pts[kc][:, ql * P:(ql + 1) * P],
                            rhs=v_sb[kc][:, h, :],
                            start=(kc == 0), stop=(kc == KB - 1))
                    rt = spool.tile([P, 1], F32, name="rt")
                    nc.vector.reciprocal(out=rt, in_=av[:, D:D + 1])
                    nc.vector.tensor_scalar_mul(
                        out=a_tiles[ql][:, h * D:(h + 1) * D],
                        in0=av[:, 0:D], scalar1=rt)
            for ql in range(8):
                at = []
                for c2 in range(MC):
                    tp = pss.tile([P, P], F32, name="pss")
                    nc.tensor.transpose(
                        tp, a_tiles[ql][:, c2 * P:(c2 + 1) * P], ident)
                    att = atpool.tile([P, P], F32R, name="att")
                    nc.vector.tensor_copy(out=att, in_=tp)
                    at.append(att)
                yt = ypool.tile([P, E], F32, name="yt")
                for nn in range(2):
                    py = ps.tile([P, 1024], F32, name="ps")[:, :512]
                    for c2 in range(MC):
                        nc.tensor.matmul(
                            py, lhsT=at[c2],
                            rhs=wo_sb[c2][:, nn * 512:(nn + 1) * 512],
                            start=(c2 == 0), stop=(c2 == MC - 1))
                    nc.vector.tensor_copy(
                        out=yt[:, nn * 512:(nn + 1) * 512], in_=py)
                q0 = qb * 1024 + ql * P
                nc.sync.dma_start(out=y[q0:q0 + P, :], in_=yt)

    _split_drain_waits(nc)
    return nc


_CACHE = {}


def _get_nc():
    if "nc" not in _CACHE:
        _CACHE["nc"] = _build()
    return _CACHE["nc"]


def kernel(query, key, value, Wq, bq, Wk, bk, Wv, bv, Wo, bo, gate):
    global LAST_RESULTS
    query = np.asarray(query, np.float32)
    key = np.asarray(key, np.float32)
    value = np.asarray(value, np.float32)
    Wq = np.asarray(Wq, np.float32)
    Wk = np.asarray(Wk, np.float32)
    Wv = np.asarray(Wv, np.float32)
    Wo = np.asarray(Wo, np.float32)
    bq = np.asarray(bq, np.float32)
    bk = np.asarray(bk, np.float32)
    bv = np.asarray(bv, np.float32)
    bo = np.asarray(bo, np.float32)
    gate = np.asarray(gate, np.float32)

    scale_h = (1.0 / (1.0 + np.exp(-gate.astype(np.float64)))
               / math.sqrt(D)).astype(np.float32)

    xq_b = [np.ascontiguousarray(query[b].T) for b in range(B)]
    xk_b = [np.ascontiguousarray(key[b].T) for b in range(B)]
    xv_b = [np.ascontiguousarray(value[b].T) for b in range(B)]

    in_maps = []
    for core in range(NCORES):
        b, g = divmod(core, GROUPS)
        rows = slice(g * DH, (g + 1) * DH)
        qs = np.repeat(scale_h[g * HG:(g + 1) * HG], D)
        in_maps.append({
            "xqT": xq_b[b], "xkT": xk_b[b], "xvT": xv_b[b],
            "wqT": np.ascontiguousarray(Wq[rows].T),
            "wkT": np.ascontiguousarray(Wk[rows].T),
            "wvT": np.ascontiguousarray(Wv[rows].T),
            "woB": np.ascontiguousarray(Wo[:, rows].T),
            "qscale": np.ascontiguousarray(qs),
            "qbias": np.ascontiguousarray(bq[rows] * qs),
            "kbias": np.ascontiguousarray(bk[rows]),
            "vbias": np.ascontiguousarray(bv[rows]),
        })

    res = run_bass_kernel_spmd(_get_nc(), in_maps, list(range(NCORES)),
                               trace=TRACE)
    LAST_RESULTS = res
    out = np.empty((B, N, E), np.float32)
    for b in range(B):
        acc = res.results[b * GROUPS]["y"].astype(np.float32).copy()
        for g in range(1, GROUPS):
            acc += res.results[b * GROUPS + g]["y"]
        out[b] = acc + bo
    return out


# revision 6
# speedup vs baseline: 1.2453x; 1.0832x over previous
"""Gated multi-head attention on 8 NeuronCores.

Sharding (hardcoded): core c -> (batch b = c // 4, head-group g = c % 4).
Data-parallel over B=2, tensor-parallel over the 16 heads in groups of 4.
Each core computes its 4 heads' attention plus the corresponding slice of
the output projection; the host sums the 4 head-group partials per batch
and adds the output bias.

Per-core kernel (all matmuls fp32r unless noted):
  qT[256,2048] = (Wq_slice.T).T @ x_q.T   (gate sigmoid/sqrt(D) + bias folded
                                           into the PSUM->SBUF eviction)
  kT[256,2048], v[2048,256] likewise (v in natural [pos, dim] layout, bf16,
                                      with a ones-column appended per head)
  per head h, per 512-wide query block:
    S^T[k,q] = kT_h.T-chunks @ qT_h      (K=64 contraction)
    P^T = exp(S^T)                       (no max-subtraction: logits ~ +-4)
    acc[q,65] = P^T-chunks.T @ [V_h | 1] (bf16; col 64 = softmax denominator)
    A[q, h*64:...] = acc[:, :64] * recip(acc[:, 64])
  y[q,1024] = A @ Wo_slice.T  via PE-transpose of A then fp32r matmul
"""

import math
from contextlib import ExitStack

import numpy as np

import concourse.bass as bass
import concourse.tile as tile
from concourse import mybir
from concourse.bass_utils import run_bass_kernel_spmd
from concourse.masks import make_identity

B = 2
N = 2048
E = 1024
H = 16
D = 64
NCORES = 8
GROUPS = NCORES // B      # head-groups per batch
HG = H // GROUPS          # heads per core
DH = HG * D               # 256 head-dims per core
P = 128

F32 = mybir.dt.float32
F32R = mybir.dt.float32r
BF16 = mybir.dt.bfloat16
AF = mybir.ActivationFunctionType

TRACE = False
LAST_RESULTS = None


def _split_drain_waits(nc):
    """The installed walrus build accepts only ONE sync-wait per instruction
    (one NEURON_ISA_TPB_EVENTS slot), but Tile emits several on drains,
    matmuls, etc.  Hoist all but the last wait onto dedicated single-wait
    NOPs ahead of the instruction on the same engine (the lowering newer
    walrus performs itself)."""
    n = 0
    for fn in nc.m.functions:
        for bb in fn.blocks:
            insts = bb.instructions
            idx = 0
            while idx < len(insts):
                inst = insts[idx]
                si = inst.sync_info
                if si is not None and len(si.on_wait) > 1:
                    waits = list(si.on_wait)
                    nops = []
                    for w in waits[:-1]:
                        n += 1
                        nop = mybir.InstNoOp(
                            name=f"waitsplit-{n}",
                            engine=inst.engine,
                            sync_info=mybir.SyncInfo(on_wait=[w], on_update=[]),
                            bass_nofuse=True,
                        )
                        nc.register_instruction(nop)
                        nops.append(nop)
                    inst.sync_info = mybir.SyncInfo(
                        on_wait=[waits[-1]], on_update=list(si.on_update))
                    insts[idx:idx] = nops
                    idx += len(nops)
                idx += 1
    return n


def _r(ap):
    return ap.bitcast(F32R)


def _build():
    nc = bass.Bass()
    xqT = nc.dram_tensor("xqT", [E, N], F32R, kind="ExternalInput")
    xkT = nc.dram_tensor("xkT", [E, N], F32R, kind="ExternalInput")
    xvT = nc.dram_tensor("xvT", [E, N], F32R, kind="ExternalInput")
    wqT = nc.dram_tensor("wqT", [E, DH], F32R, kind="ExternalInput")
    wkT = nc.dram_tensor("wkT", [E, DH], F32R, kind="ExternalInput")
    wvT = nc.dram_tensor("wvT", [E, DH], F32R, kind="ExternalInput")
    woB = nc.dram_tensor("woB", [DH, E], F32R, kind="ExternalInput")
    qscale = nc.dram_tensor("qscale", [DH], F32, kind="ExternalInput")
    qbias = nc.dram_tensor("qbias", [DH], F32, kind="ExternalInput")
    kbias = nc.dram_tensor("kbias", [DH], F32, kind="ExternalInput")
    vbias = nc.dram_tensor("vbias", [DH], F32, kind="ExternalInput")
    y = nc.dram_tensor("y", [N, E], F32, kind="ExternalOutput")

    KC = E // P            # 8 contraction chunks over the embed dim
    MC = DH // P           # 2 partition chunks over this core's head dims
    NB = N // 512          # 4 query blocks
    KB = N // P            # 16 key-position chunks

    with ExitStack() as ctx:
        tc = ctx.enter_context(tile.TileContext(nc))
        const = ctx.enter_context(tc.tile_pool(name="const", bufs=1))
        xpool = ctx.enter_context(tc.tile_pool(name="xpool", bufs=KC))
        wpool = ctx.enter_context(tc.tile_pool(name="wpool", bufs=KC))
        wopool = ctx.enter_context(tc.tile_pool(name="wopool", bufs=MC))
        qkpool = ctx.enter_context(tc.tile_pool(name="qkpool", bufs=MC))
        vpool = ctx.enter_context(tc.tile_pool(name="vpool", bufs=KB))
        ptpool = ctx.enter_context(tc.tile_pool(name="ptpool", bufs=18))
        apool = ctx.enter_context(tc.tile_pool(name="apool", bufs=10))
        atpool = ctx.enter_context(tc.tile_pool(name="atpool", bufs=4))
        ypool = ctx.enter_context(tc.tile_pool(name="ypool", bufs=2))
        spool = ctx.enter_context(tc.tile_pool(name="spool", bufs=8))
        pp = ctx.enter_context(tc.tile_pool(name="pp", bufs=2, space="PSUM"))
        stq = ctx.enter_context(tc.tile_pool(name="stq", bufs=2, space="PSUM"))
        pss = ctx.enter_context(tc.tile_pool(name="pss", bufs=2, space="PSUM"))

        ident = const.tile([P, P], F32)
        make_identity(nc, ident)

        qs_sb = const.tile([P, MC], F32, name="qs")
        nc.sync.dma_start(out=qs_sb, in_=qscale[:].rearrange("(c p) -> p c", p=P))
        qb_sb = const.tile([P, MC], F32, name="qb")
        nc.sync.dma_start(out=qb_sb, in_=qbias[:].rearrange("(c p) -> p c", p=P))
        kb_sb = const.tile([P, MC], F32, name="kb")
        nc.sync.dma_start(out=kb_sb, in_=kbias[:].rearrange("(c p) -> p c", p=P))
        vb_ap = vbias[:]
        vb_bc = const.tile([P, DH], F32, name="vb")
        nc.gpsimd.dma_start(out=vb_bc, in_=bass.AP(
            tensor=vb_ap.tensor, offset=vb_ap.offset, ap=[[0, P]] + vb_ap.ap))

        def load_w(w_dram, tag):
            chunks = []
            for kc in range(KC):
                t = wpool.tile([P, DH], F32R, name=tag)
                nc.sync.dma_start(out=t, in_=w_dram[kc * P:(kc + 1) * P, :])
                chunks.append(t)
            return chunks

        wq_c = load_w(wqT, "wq")
        wk_c = load_w(wkT, "wk")
        wv_c = load_w(wvT, "wv")
        wo_sb = []
        for c in range(MC):
            t = wopool.tile([P, E], F32R, name="wo")
            nc.sync.dma_start(out=t, in_=woB[c * P:(c + 1) * P, :])
            wo_sb.append(t)

        def load_x(x_dram):
            xs = []
            for kc in range(KC):
                t = xpool.tile([P, N], F32R, name="xs")
                nc.sync.dma_start(out=t, in_=x_dram[kc * P:(kc + 1) * P, :])
                xs.append(t)
            return xs

        # --- transposed projections: out[c][dd, n] ---
        def proj_T(xs, w_c, tag, scale_sb, bias_sb):
            outs = []
            for c in range(MC):
                o = qkpool.tile([P, N], BF16, name=tag)
                outs.append(o)
                for nb in range(NB):
                    pt = pp.tile([P, 512], F32, name="pp")
                    for kc in range(KC):
                        nc.tensor.matmul(
                            pt,
                            lhsT=w_c[kc][:, c * P:(c + 1) * P],
                            rhs=xs[kc][:, nb * 512:(nb + 1) * 512],
                            start=(kc == 0), stop=(kc == KC - 1))
                    nc.scalar.activation(
                        out=o[:, nb * 512:(nb + 1) * 512], in_=pt,
                        func=AF.Identity,
                        bias=bias_sb[:, c:c + 1],
                        scale=scale_sb[:, c:c + 1] if scale_sb is not None else 1.0)
            return outs

        xk = load_x(xkT)
        kT = proj_T(xk, wk_c, "kt", None, kb_sb)
        xq = load_x(xqT)
        qT = proj_T(xq, wq_c, "qt", qs_sb, qb_sb)

        # --- v in natural [pos, dim] layout, bf16, ones column at dim 64 ---
        xv = load_x(xvT)
        v_sb = []
        for m in range(KB):
            vt = vpool.tile([P, HG, D + 1], BF16, name="vt")
            nc.gpsimd.memset(vt[:, :, D:D + 1], 1.0)
            pv = pp.tile([P, 512], F32, name="pp")[:, :DH]
            for kc in range(KC):
                nc.tensor.matmul(
                    pv,
                    lhsT=xv[kc][:, m * P:(m + 1) * P],
                    rhs=wv_c[kc],
                    start=(kc == 0), stop=(kc == KC - 1))
            nc.vector.tensor_add(
                out=vt[:, :, 0:D],
                in0=pv.rearrange("p (h d) -> p h d", h=HG),
                in1=vb_bc.rearrange("p (h d) -> p h d", h=HG))
            v_sb.append(vt)

        # --- attention + output projection, one 1024-wide query block at a time
        for qb in range(N // 1024):
            a_tiles = [apool.tile([P, DH], F32, name="acc") for _ in range(8)]
            for h in range(HG):
                c, off = divmod(h * D, P)
                pts = []
                for kc in range(KB):
                    stp = stq.tile([P, 1024], F32, name="stq")
                    for qh in range(2):
                        nc.tensor.matmul(
                            stp[:, qh * 512:(qh + 1) * 512],
                            lhsT=kT[c][off:off + D, kc * P:(kc + 1) * P],
                            rhs=qT[c][off:off + D,
                                      qb * 1024 + qh * 512:qb * 1024 + (qh + 1) * 512],
                            start=True, stop=True)
                    ptile = ptpool.tile([P, 1024], BF16, name="pt")
                    nc.scalar.activation(out=ptile, in_=stp, func=AF.Exp)
                    pts.append(ptile)
                for ql in range(8):
                    av = pss.tile([P, D + 1], F32, name="pss")
                    for kc in range(KB):
                        nc.tensor.matmul(
                            av,
                            lhsT=pts[kc][:, ql * P:(ql + 1) * P],
                            rhs=v_sb[kc][:, h, :],
                            start=(kc == 0), stop=(kc == KB - 1))
                    rt = spool.tile([P, 1], F32, name="rt")
                    nc.vector.reciprocal(out=rt, in_=av[:, D:D + 1])
                    nc.vector.tensor_scalar_mul(
                        out=a_tiles[ql][:, h * D:(h + 1) * D],
                        in0=av[:, 0:D], scalar1=rt)
            for ql in range(8):
                at = []
                for c2 in range(MC):
                    tp = pss.tile([P, P], F32, name="pss")
                    nc.tensor.transpose(
                        tp, a_tiles[ql][:, c2 * P:(c2 + 1) * P], ident)
                    att = atpool.tile([P, P], F32R, name="att")
                    nc.vector.tensor_copy(out=att, in_=tp)
                    at.append(att)
                yt = ypool.tile([P, E], F32, name="yt")
                for nn in range(2):
                    py = pp.tile([P, 512], F32, name="pp")
                    for c2 in range(MC):
                        nc.tensor.matmul(
                            py, lhsT=at[c2],
                            rhs=wo_sb[c2][:, nn * 512:(nn + 1) * 512],
                            start=(c2 == 0), stop=(c2 == MC - 1))
                    nc.vector.tensor_copy(
                        out=yt[:, nn * 512:(nn + 1) * 512], in_=py)
                q0 = qb * 1024 + ql * P
                nc.sync.dma_start(out=y[q0:q0 + P, :], in_=yt)

    _split_drain_waits(nc)
    return nc


_CACHE = {}


def _get_nc():
    if "nc" not in _CACHE:
        _CACHE["nc"] = _build()
    return _CACHE["nc"]


def kernel(query, key, value, Wq, bq, Wk, bk, Wv, bv, Wo, bo, gate):
    global LAST_RESULTS
    query = np.asarray(query, np.float32)
    key = np.asarray(key, np.float32)
    value = np.asarray(value, np.float32)
    Wq = np.asarray(Wq, np.float32)
    Wk = np.asarray(Wk, np.float32)
    Wv = np.asarray(Wv, np.float32)
    Wo = np.asarray(Wo, np.float32)
    bq = np.asarray(bq, np.float32)
    bk = np.asarray(bk, np.float32)
    bv = np.asarray(bv, np.float32)
    bo = np.asarray(bo, np.float32)
    gate = np.asarray(gate, np.float32)

    scale_h = (1.0 / (1.0 + np.exp(-gate.astype(np.float64)))
               / math.sqrt(D)).astype(np.float32)

    xq_b = [np.ascontiguousarray(query[b].T) for b in range(B)]
    xk_b = [np.ascontiguousarray(key[b].T) for b in range(B)]
    xv_b = [np.ascontiguousarray(value[b].T) for b in range(B)]

    in_maps = []
    for core in range(NCORES):
        b, g = divmod(core, GROUPS)
        rows = slice(g * DH, (g + 1) * DH)
        qs = np.repeat(scale_h[g * HG:(g + 1) * HG], D)
        in_maps.append({
            "xqT": xq_b[b], "xkT": xk_b[b], "xvT": xv_b[b],
            "wqT": np.ascontiguousarray(Wq[rows].T),
            "wkT": np.ascontiguousarray(Wk[rows].T),
            "wvT": np.ascontiguousarray(Wv[rows].T),
            "woB": np.ascontiguousarray(Wo[:, rows].T),
            "qscale": np.ascontiguousarray(qs),
            "qbias": np.ascontiguousarray(bq[rows] * qs),
            "kbias": np.ascontiguousarray(bk[rows]),
            "vbias": np.ascontiguousarray(bv[rows]),
        })

    res = run_bass_kernel_spmd(_get_nc(), in_maps, list(range(NCORES)),
                               trace=TRACE)
    LAST_RESULTS = res
    out = np.empty((B, N, E), np.float32)
    for b in range(B):
        acc = res.results[b * GROUPS]["y"].astype(np.float32).copy()
        for g in range(1, GROUPS):
            acc += res.results[b * GROUPS + g]["y"]
        out[b] = acc + bo
    return out


# revision 10
# speedup vs baseline: 1.4550x; 1.1685x over previous
"""Gated multi-head attention on 8 NeuronCores.

Sharding (hardcoded): core c -> (batch b = c // 4, head-group g = c % 4).
Data-parallel over B=2, tensor-parallel over the 16 heads in groups of 4.
Each core computes its 4 heads' attention plus the corresponding slice of
the output projection; the host sums the 4 head-group partials per batch
and adds the output bias.

Per-core kernel (all matmuls fp32r unless noted):
  qT[256,2048] = (Wq_slice.T).T @ x_q.T   (gate sigmoid/sqrt(D) + bias folded
                                           into the PSUM->SBUF eviction)
  kT[256,2048], v[2048,256] likewise (v in natural [pos, dim] layout, bf16,
                                      with a ones-column appended per head)
  per head h, per 512-wide query block:
    S^T[k,q] = kT_h.T-chunks @ qT_h      (K=64 contraction)
    P^T = exp(S^T)                       (no max-subtraction: logits ~ +-4)
    acc[q,65] = P^T-chunks.T @ [V_h | 1] (bf16; col 64 = softmax denominator)
    A[q, h*64:...] = acc[:, :64] * recip(acc[:, 64])
  y[q,1024] = A @ Wo_slice.T  via PE-transpose of A then fp32r matmul
"""

import math
from contextlib import ExitStack

import numpy as np

import concourse.bass as bass
import concourse.tile as tile
from concourse import mybir
from concourse.bass_utils import run_bass_kernel_spmd
from concourse.masks import make_identity

B = 2
N = 2048
E = 1024
H = 16
D = 64
NCORES = 8
GROUPS = NCORES // B      # head-groups per batch
HG = H // GROUPS          # heads per core
DH = HG * D               # 256 head-dims per core
P = 128

F32 = mybir.dt.float32
F32R = mybir.dt.float32r
BF16 = mybir.dt.bfloat16
AF = mybir.ActivationFunctionType

TRACE = False
LAST_RESULTS = None


def _split_drain_waits(nc):
    """The installed walrus build accepts only ONE sync-wait per instruction
    (one NEURON_ISA_TPB_EVENTS slot), but Tile emits several on drains,
    matmuls, etc.  Hoist all but the last wait onto dedicated single-wait
    NOPs ahead of the instruction on the same engine (the lowering newer
    walrus performs itself)."""
    n = 0
    for fn in nc.m.functions:
        for bb in fn.blocks:
            insts = bb.instructions
            idx = 0
            while idx < len(insts):
                inst = insts[idx]
                si = inst.sync_info
                if si is not None and len(si.on_wait) > 1:
                    waits = list(si.on_wait)
                    nops = []
                    for w in waits[:-1]:
                        n += 1
                        nop = mybir.InstNoOp(
                            name=f"waitsplit-{n}",
                            engine=inst.engine,
                            sync_info=mybir.SyncInfo(on_wait=[w], on_update=[]),
                            bass_nofuse=True,
                        )
                        nc.register_instruction(nop)
                        nops.append(nop)
                    inst.sync_info = mybir.SyncInfo(
                        on_wait=[waits[-1]], on_update=list(si.on_update))
                    insts[idx:idx] = nops
                    idx += len(nops)
                idx += 1
    return n


def _r(ap):
    return ap.bitcast(F32R)


def _build():
    nc = bass.Bass()
    xqT = nc.dram_tensor("xqT", [E, N], BF16, kind="ExternalInput")
    xkT = nc.dram_tensor("xkT", [E, N], BF16, kind="ExternalInput")
    xvT = nc.dram_tensor("xvT", [E, N], BF16, kind="ExternalInput")
    wqT = nc.dram_tensor("wqT", [E, DH], BF16, kind="ExternalInput")
    wkT = nc.dram_tensor("wkT", [E, DH], BF16, kind="ExternalInput")
    wvT = nc.dram_tensor("wvT", [E, DH], BF16, kind="ExternalInput")
    woB = nc.dram_tensor("woB", [DH, E], BF16, kind="ExternalInput")
    qscale = nc.dram_tensor("qscale", [DH], F32, kind="ExternalInput")
    qbias = nc.dram_tensor("qbias", [DH], F32, kind="ExternalInput")
    kbias = nc.dram_tensor("kbias", [DH], F32, kind="ExternalInput")
    vbias = nc.dram_tensor("vbias", [DH], F32, kind="ExternalInput")
    y = nc.dram_tensor("y", [N, E], F32, kind="ExternalOutput")

    KC = E // P            # 8 contraction chunks over the embed dim
    MC = DH // P           # 2 partition chunks over this core's head dims
    NB = N // 512          # 4 query blocks
    KB = N // P            # 16 key-position chunks

    with ExitStack() as ctx:
        tc = ctx.enter_context(tile.TileContext(nc))
        const = ctx.enter_context(tc.tile_pool(name="const", bufs=1))
        xpool = ctx.enter_context(tc.tile_pool(name="xpool", bufs=12))
        wpool = ctx.enter_context(tc.tile_pool(name="wpool", bufs=KC))
        wopool = ctx.enter_context(tc.tile_pool(name="wopool", bufs=MC))
        qkpool = ctx.enter_context(tc.tile_pool(name="qkpool", bufs=MC))
        vpool = ctx.enter_context(tc.tile_pool(name="vpool", bufs=KB))
        ptpool = ctx.enter_context(tc.tile_pool(name="ptpool", bufs=18))
        apool = ctx.enter_context(tc.tile_pool(name="apool", bufs=10))
        atpool = ctx.enter_context(tc.tile_pool(name="atpool", bufs=4))
        ypool = ctx.enter_context(tc.tile_pool(name="ypool", bufs=2))
        spool = ctx.enter_context(tc.tile_pool(name="spool", bufs=8))
        pp = ctx.enter_context(tc.tile_pool(name="pp", bufs=2, space="PSUM"))
        stq = ctx.enter_context(tc.tile_pool(name="stq", bufs=2, space="PSUM"))
        pss = ctx.enter_context(tc.tile_pool(name="pss", bufs=2, space="PSUM"))

        ident = const.tile([P, P], F32)
        make_identity(nc, ident)

        qs_sb = const.tile([P, MC], F32, name="qs")
        nc.sync.dma_start(out=qs_sb, in_=qscale[:].rearrange("(c p) -> p c", p=P))
        qb_sb = const.tile([P, MC], F32, name="qb")
        nc.sync.dma_start(out=qb_sb, in_=qbias[:].rearrange("(c p) -> p c", p=P))
        kb_sb = const.tile([P, MC], F32, name="kb")
        nc.sync.dma_start(out=kb_sb, in_=kbias[:].rearrange("(c p) -> p c", p=P))
        vb_ap = vbias[:]
        vb_bc = const.tile([P, DH], F32, name="vb")
        nc.gpsimd.dma_start(out=vb_bc, in_=bass.AP(
            tensor=vb_ap.tensor, offset=vb_ap.offset, ap=[[0, P]] + vb_ap.ap))

        def load_w(w_dram, tag):
            chunks = []
            for kc in range(KC):
                t = wpool.tile([P, DH], BF16, name=tag)
                nc.sync.dma_start(out=t, in_=w_dram[kc * P:(kc + 1) * P, :])
                chunks.append(t)
            return chunks

        def load_x(x_dram):
            xs = []
            for kc in range(KC):
                t = xpool.tile([P, N], BF16, name="xs")
                nc.sync.dma_start(out=t, in_=x_dram[kc * P:(kc + 1) * P, :])
                xs.append(t)
            return xs

        # --- transposed projections: out[c][dd, n] ---
        def proj_T(xs, w_c, tag, scale_sb, bias_sb):
            outs = []
            for c in range(MC):
                o = qkpool.tile([P, N], BF16, name=tag)
                outs.append(o)
                for nb in range(NB):
                    pt = pp.tile([P, 512], F32, name="pp")
                    for kc in range(KC):
                        nc.tensor.matmul(
                            pt,
                            lhsT=w_c[kc][:, c * P:(c + 1) * P],
                            rhs=xs[kc][:, nb * 512:(nb + 1) * 512],
                            start=(kc == 0), stop=(kc == KC - 1))
                    nc.scalar.activation(
                        out=o[:, nb * 512:(nb + 1) * 512], in_=pt,
                        func=AF.Identity,
                        bias=bias_sb[:, c:c + 1],
                        scale=scale_sb[:, c:c + 1] if scale_sb is not None else 1.0)
            return outs

        wk_c = load_w(wkT, "wk")
        xk = load_x(xkT)
        kT = proj_T(xk, wk_c, "kt", None, kb_sb)
        wq_c = load_w(wqT, "wq")
        xq = load_x(xqT)
        qT = proj_T(xq, wq_c, "qt", qs_sb, qb_sb)
        wo_sb = []
        for c in range(MC):
            t = wopool.tile([P, E], BF16, name="wo")
            nc.sync.dma_start(out=t, in_=woB[c * P:(c + 1) * P, :])
            wo_sb.append(t)

        # --- v in natural [pos, dim] layout, bf16, ones column at dim 64 ---
        wv_c = load_w(wvT, "wv")
        xv = load_x(xvT)
        v_sb = []
        for m in range(KB):
            vt = vpool.tile([P, HG, D + 1], BF16, name="vt")
            nc.gpsimd.memset(vt[:, :, D:D + 1], 1.0)
            pv = pp.tile([P, 512], F32, name="pp")[:, :DH]
            for kc in range(KC):
                nc.tensor.matmul(
                    pv,
                    lhsT=xv[kc][:, m * P:(m + 1) * P],
                    rhs=wv_c[kc],
                    start=(kc == 0), stop=(kc == KC - 1))
            nc.vector.tensor_add(
                out=vt[:, :, 0:D],
                in0=pv.rearrange("p (h d) -> p h d", h=HG),
                in1=vb_bc.rearrange("p (h d) -> p h d", h=HG))
            v_sb.append(vt)

        # --- attention + output projection, one 1024-wide query block at a time
        for qb in range(N // 1024):
            a_tiles = [apool.tile([P, DH], F32, name="acc") for _ in range(8)]
            for h in range(HG):
                c, off = divmod(h * D, P)
                pts = []
                for kc in range(KB):
                    stp = stq.tile([P, 1024], F32, name="stq")
                    for qh in range(2):
                        nc.tensor.matmul(
                            stp[:, qh * 512:(qh + 1) * 512],
                            lhsT=kT[c][off:off + D, kc * P:(kc + 1) * P],
                            rhs=qT[c][off:off + D,
                                      qb * 1024 + qh * 512:qb * 1024 + (qh + 1) * 512],
                            start=True, stop=True)
                    ptile = ptpool.tile([P, 1024], BF16, name="pt")
                    nc.scalar.activation(out=ptile, in_=stp, func=AF.Exp)
                    pts.append(ptile)
                for ql in range(8):
                    av = pss.tile([P, D + 1], F32, name="pss")
                    for kc in range(KB):
                        nc.tensor.matmul(
                            av,
                            lhsT=pts[kc][:, ql * P:(ql + 1) * P],
                            rhs=v_sb[kc][:, h, :],
                            start=(kc == 0), stop=(kc == KB - 1))
                    rt = spool.tile([P, 1], F32, name="rt")
                    nc.vector.reciprocal(out=rt, in_=av[:, D:D + 1])
                    nc.vector.tensor_scalar_mul(
                        out=a_tiles[ql][:, h * D:(h + 1) * D],
                        in0=av[:, 0:D], scalar1=rt)
            for ql in range(8):
                at = []
                for c2 in range(MC):
                    tp = pss.tile([P, P], F32, name="pss")
                    nc.tensor.transpose(
                        tp, a_tiles[ql][:, c2 * P:(c2 + 1) * P], ident)
                    att = atpool.tile([P, P], BF16, name="att")
                    nc.vector.tensor_copy(out=att, in_=tp)
                    at.append(att)
                yt = ypool.tile([P, E], F32, name="yt")
                for nn in range(2):
                    py = pp.tile([P, 512], F32, name="pp")
                    for c2 in range(MC):
                        nc.tensor.matmul(
                            py, lhsT=at[c2],
                            rhs=wo_sb[c2][:, nn * 512:(nn + 1) * 512],
                            start=(c2 == 0), stop=(c2 == MC - 1))
                    nc.vector.tensor_copy(
                        out=yt[:, nn * 512:(nn + 1) * 512], in_=py)
                q0 = qb * 1024 + ql * P
                nc.sync.dma_start(out=y[q0:q0 + P, :], in_=yt)

    _split_drain_waits(nc)
    return nc


_CACHE = {}


def _get_nc():
    if "nc" not in _CACHE:
        _CACHE["nc"] = _build()
    return _CACHE["nc"]


def kernel(query, key, value, Wq, bq, Wk, bk, Wv, bv, Wo, bo, gate):
    global LAST_RESULTS
    query = np.asarray(query, np.float32)
    key = np.asarray(key, np.float32)
    value = np.asarray(value, np.float32)
    Wq = np.asarray(Wq, np.float32)
    Wk = np.asarray(Wk, np.float32)
    Wv = np.asarray(Wv, np.float32)
    Wo = np.asarray(Wo, np.float32)
    bq = np.asarray(bq, np.float32)
    bk = np.asarray(bk, np.float32)
    bv = np.asarray(bv, np.float32)
    bo = np.asarray(bo, np.float32)
    gate = np.asarray(gate, np.float32)

    scale_h = (1.0 / (1.0 + np.exp(-gate.astype(np.float64)))
               / math.sqrt(D)).astype(np.float32)

    xq_b = [np.ascontiguousarray(query[b].T) for b in range(B)]
    xk_b = [np.ascontiguousarray(key[b].T) for b in range(B)]
    xv_b = [np.ascontiguousarray(value[b].T) for b in range(B)]

    in_maps = []
    for core in range(NCORES):
        b, g = divmod(core, GROUPS)
        rows = slice(g * DH, (g + 1) * DH)
        qs = np.repeat(scale_h[g * HG:(g + 1) * HG], D)
        in_maps.append({
            "xqT": xq_b[b], "xkT": xk_b[b], "xvT": xv_b[b],
            "wqT": np.ascontiguousarray(Wq[rows].T),
            "wkT": np.ascontiguousarray(Wk[rows].T),
            "wvT": np.ascontiguousarray(Wv[rows].T),
            "woB": np.ascontiguousarray(Wo[:, rows].T),
            "qscale": np.ascontiguousarray(qs),
            "qbias": np.ascontiguousarray(bq[rows] * qs),
            "kbias": np.ascontiguousarray(bk[rows]),
            "vbias": np.ascontiguousarray(bv[rows]),
        })

    from concourse import mybir as _mb
    bf = _mb.dt.np(_mb.dt.bfloat16)
    for m in in_maps:
        for k in ("xqT", "xkT", "xvT", "wqT", "wkT", "wvT", "woB"):
            m[k] = m[k].astype(bf)
    res = run_bass_kernel_spmd(_get_nc(), in_maps, list(range(NCORES)),
                               trace=TRACE)
    LAST_RESULTS = res
    out = np.empty((B, N, E), np.float32)
    for b in range(B):
        acc = res.results[b * GROUPS]["y"].astype(np.float32).copy()
        for g in range(1, GROUPS):
            acc += res.results[b * GROUPS + g]["y"]
        out[b] = acc + bo
    return out
